# revision 4
# baseline (speedup 1.0000x reference)
"""CLOCs pairwise-IoU association kernel for Trainium2 (8 NeuronCores).

Problem: boxes [N=10000,4], query_boxes [K=500,4] -> dense association tensor
  overlaps     [K*N, 4] f32 : (iou|-10, s3d, s2d|-10, dis) per (k,n) pair
  tensor_index [K*N, 2] i32 : (k, n)
  valid        [K*N]   bool : iw>0 & ih>0

Sharding: N split into 8 contiguous strips of 1250 (one per core); every core
holds all K=500 query boxes on the partition axis (4 k-tiles of <=128).
N-side per-box rows (coords/areas/scores) are partition-broadcast via exact
fp32 TensorE matmuls with a ones[1,128] weight; K-side per-box values are
per-partition scalars fed to tensor_scalar / scalar_tensor_tensor ops.

Per k-tile dataflow (free dim = 1250 n's):
  DVE : t2=max(bx1,qx1); iw=min(bx2,qx2)-t2; t4=max(by1,qy1); ih=min(by2,qy2)-t4
        inter=max(iw,0)*relu(ih); ua=UA0-inter; r~=1/ua; vd=(inter<=0)*-10
        iou=inter*r; out[...,0]=iou+vd
  ACT : relu(ih); out[...,1|3]=s3dis copy (strided); out[...,2]=scale*vd+bias
        valid=sign(inter) (u8)
  PE  : UA0 = box_area[n] + qbox_area[k] (fused in the broadcast matmul)
  POOL: index even slots = k (per-partition const); odd slots broadcast once
"""

import numpy as np

N = 10000
K = 500
NCORES = 8
NLOC = N // NCORES          # 1250
KT = 4                      # k-tiles of 128 (last has 116 real rows)
NEG = np.float32(-10.0)

_kernel_cache = {}


def _build_kernel(criterion: int):
    import concourse.bacc as bacc
    import concourse.tile as tile
    from concourse import mybir

    f32 = mybir.dt.float32
    i32 = mybir.dt.int32
    u8 = mybir.dt.uint8
    Alu = mybir.AluOpType
    Act = mybir.ActivationFunctionType

    nc = bacc.Bacc("TRN2", target_bir_lowering=False, debug=False,
                   num_devices=NCORES)

    A = nc.dram_tensor("A", [8, 2 * NLOC], f32, kind="ExternalInput").ap()
    Q = nc.dram_tensor("Q", [128, 8 * KT], f32, kind="ExternalInput").ap()
    WUA = nc.dram_tensor("WUA", [2, 128 * KT], f32, kind="ExternalInput").ap()
    WON = nc.dram_tensor("WON", [1, 128], f32, kind="ExternalInput").ap()

    OV = nc.dram_tensor("OV", [K, 4 * NLOC], f32, kind="ExternalOutput").ap()
    IX = nc.dram_tensor("IX", [K, 2 * NLOC], i32, kind="ExternalOutput").ap()
    VA = nc.dram_tensor("VA", [K, NLOC], u8, kind="ExternalOutput").ap()

    # criterion==-1 subtracts inter from the union denominator
    gamma = 1.0 if criterion == -1 else 0.0

    def chunks(total, step=512):
        c0 = 0
        while c0 < total:
            yield c0, min(step, total - c0)
            c0 += step

    with tile.TileContext(nc) as tc:
        with (
            tc.tile_pool(name="const", bufs=1) as const,
            tc.tile_pool(name="tmp1", bufs=1) as tmp1,
            tc.tile_pool(name="tmp2", bufs=2) as tmp2,
            tc.tile_pool(name="outp", bufs=2) as outp,
            tc.tile_pool(name="ps", bufs=2, space="PSUM") as ps,
        ):
            # each broadcast-source row in its own tile (matmul rhs must
            # start at base partition 0); AB = [barea; ones] for the fused
            # UA0 = alpha*barea + w[k] matmul
            R_bx1 = const.tile([1, NLOC], f32, tag="R_bx1")
            R_bx2 = const.tile([1, NLOC], f32, tag="R_bx2")
            R_by1 = const.tile([1, NLOC], f32, tag="R_by1")
            R_by2 = const.tile([1, NLOC], f32, tag="R_by2")
            R_s3d = const.tile([1, 2 * NLOC], f32, tag="R_s3d")
            R_idx = const.tile([1, 2 * NLOC], f32, tag="R_idx")
            AB = const.tile([2, NLOC], f32, tag="AB")
            Q_sb = const.tile([128, 8 * KT], f32)
            WUA_sb = const.tile([2, 128 * KT], f32)
            WON_sb = const.tile([1, 128], f32)
            nc.sync.dma_start(out=R_bx1, in_=A[0:1, 0:NLOC])
            nc.sync.dma_start(out=R_bx2, in_=A[1:2, 0:NLOC])
            nc.sync.dma_start(out=R_by1, in_=A[2:3, 0:NLOC])
            nc.sync.dma_start(out=R_by2, in_=A[3:4, 0:NLOC])
            nc.sync.dma_start(out=AB[0:1, :], in_=A[4:5, 0:NLOC])
            nc.sync.dma_start(out=AB[1:2, :], in_=A[5:6, 0:NLOC])
            nc.sync.dma_start(out=R_s3d, in_=A[6:7, :])
            nc.sync.dma_start(out=R_idx, in_=A[7:8, :])
            nc.sync.dma_start(out=Q_sb, in_=Q)
            nc.sync.dma_start(out=WUA_sb, in_=WUA)
            nc.sync.dma_start(out=WON_sb, in_=WON)

            # --- setup: partition-broadcast the n-side rows (exact) ---
            BX1 = const.tile([128, NLOC], f32, tag="BX1")
            BX2 = const.tile([128, NLOC], f32, tag="BX2")
            BY1 = const.tile([128, NLOC], f32, tag="BY1")
            BY2 = const.tile([128, NLOC], f32, tag="BY2")
            S3D = const.tile([128, 2 * NLOC], f32, tag="S3D")
            IDXB = const.tile([128, NLOC, 2], i32, tag="IDXB")

            for src, dst in ((R_bx1, BX1), (R_bx2, BX2),
                             (R_by1, BY1), (R_by2, BY2)):
                for c0, cs in chunks(NLOC):
                    pb = ps.tile([128, 512], f32, tag="pb")
                    nc.tensor.matmul(pb[:, :cs], WON_sb,
                                     src[:, c0:c0 + cs],
                                     start=True, stop=True)
                    nc.scalar.copy(dst[:, c0:c0 + cs], pb[:, :cs])
            for c0, cs in chunks(2 * NLOC):
                pb = ps.tile([128, 512], f32, tag="pb")
                nc.tensor.matmul(pb[:, :cs], WON_sb,
                                 R_s3d[:, c0:c0 + cs], start=True, stop=True)
                nc.scalar.copy(S3D[:, c0:c0 + cs], pb[:, :cs])
            IDXF = IDXB.rearrange("p f c -> p (f c)")
            for c0, cs in chunks(2 * NLOC):
                pb = ps.tile([128, 512], f32, tag="pb")
                nc.tensor.matmul(pb[:, :cs], WON_sb,
                                 R_idx[:, c0:c0 + cs], start=True, stop=True)
                nc.scalar.activation(IDXF[:, c0:c0 + cs], pb[:, :cs], Act.Copy)

            # --- steady: one pass per k-tile ---
            for t in range(KT):
                psz = 128 if t < KT - 1 else K - 128 * (KT - 1)

                def q(j, t=t):
                    return Q_sb[:, 8 * t + j:8 * t + j + 1]

                UA0 = ps.tile([128, NLOC], f32, tag="ua")
                for c0, cs in chunks(NLOC):
                    nc.tensor.matmul(UA0[:, c0:c0 + cs],
                                     WUA_sb[:, 128 * t:128 * t + 128],
                                     AB[:, c0:c0 + cs],
                                     start=True, stop=True)

                t2 = tmp1.tile([128, NLOC], f32, tag="t2")
                iw = tmp1.tile([128, NLOC], f32, tag="iw")
                t4 = tmp1.tile([128, NLOC], f32, tag="t4")
                ih = tmp2.tile([128, NLOC], f32, tag="ih")
                ihp = tmp2.tile([128, NLOC], f32, tag="ihp")
                inter = tmp2.tile([128, NLOC], f32, tag="inter")
                ua = tmp1.tile([128, NLOC], f32, tag="uat")
                r = tmp1.tile([128, NLOC], f32, tag="r")
                vd = tmp2.tile([128, NLOC], f32, tag="vd")
                iou = tmp1.tile([128, NLOC], f32, tag="iou")
                T = outp.tile([128, NLOC, 4], f32, tag="T")
                V = outp.tile([128, NLOC], u8, tag="V")

                nc.vector.tensor_scalar_max(t2, BX1, q(0))
                nc.vector.scalar_tensor_tensor(iw, BX2, q(2), t2,
                                               Alu.min, Alu.subtract)
                nc.vector.tensor_scalar_max(t4, BY1, q(1))
                nc.vector.scalar_tensor_tensor(ih, BY2, q(3), t4,
                                               Alu.min, Alu.subtract)
                nc.scalar.activation(ihp, ih, Act.Relu)
                nc.vector.scalar_tensor_tensor(inter, iw, 0.0, ihp,
                                               Alu.max, Alu.mult)
                nc.vector.scalar_tensor_tensor(ua, inter, -gamma, UA0,
                                               Alu.mult, Alu.add)
                nc.vector.reciprocal_approx_fast(r, ua)
                nc.vector.tensor_scalar(vd, inter, 0.0, -10.0,
                                        Alu.is_le, Alu.mult)
                nc.vector.tensor_mul(iou, inter, r)
                nc.vector.tensor_add(T[:, :, 0], iou, vd)
                nc.scalar.activation(T[:, :, 1::2],
                                     S3D.rearrange("p (f c) -> p f c", c=2),
                                     Act.Copy)
                nc.scalar.activation(T[:, :, 2], vd, Act.Identity,
                                     bias=q(4), scale=q(5))
                nc.scalar.activation(V, inter, Act.Sign)
                # tensor_index even slots: per-partition k constant
                nc.gpsimd.tensor_scalar(IDXB[:, :, 0], IDXB[:, :, 1],
                                        0.0, q(6), Alu.mult, Alu.add)

                r0 = 128 * t
                nc.sync.dma_start(out=OV[r0:r0 + psz, :],
                                  in_=T[:psz].rearrange("p f c -> p (f c)"))
                nc.sync.dma_start(out=IX[r0:r0 + psz, :], in_=IDXF[:psz])
                nc.sync.dma_start(out=VA[r0:r0 + psz, :], in_=V[:psz])

    nc.compile()
    return nc


def _host_prep(boxes, query_boxes, scores_3d, scores_2d, dis_to_lidar_3d,
               criterion):
    boxes = np.ascontiguousarray(boxes, dtype=np.float32)
    qb = np.ascontiguousarray(query_boxes, dtype=np.float32)
    s3d = np.ascontiguousarray(scores_3d, dtype=np.float32).reshape(-1)
    s2d = np.ascontiguousarray(scores_2d, dtype=np.float32).reshape(-1)
    dis = np.ascontiguousarray(dis_to_lidar_3d, dtype=np.float32).reshape(-1)

    barea = (boxes[:, 2] - boxes[:, 0]) * (boxes[:, 3] - boxes[:, 1])
    qarea = (qb[:, 2] - qb[:, 0]) * (qb[:, 3] - qb[:, 1])

    # K-side per-partition scalars, padded to 4*128 rows
    KP = 128 * KT
    qx1 = np.zeros(KP, np.float32); qy1 = np.zeros(KP, np.float32)
    qx2 = np.ones(KP, np.float32);  qy2 = np.ones(KP, np.float32)
    s2dp = np.zeros(KP, np.float32)
    qap = np.ones(KP, np.float32)
    qx1[:K], qy1[:K], qx2[:K], qy2[:K] = qb[:, 0], qb[:, 1], qb[:, 2], qb[:, 3]
    s2dp[:K] = s2d
    qap[:K] = qarea

    Q = np.zeros((128, 8 * KT), np.float32)
    kk = np.arange(KP, dtype=np.float32)
    for t in range(KT):
        sl = slice(128 * t, 128 * (t + 1))
        Q[:, 8 * t + 0] = qx1[sl]
        Q[:, 8 * t + 1] = qy1[sl]
        Q[:, 8 * t + 2] = qx2[sl]
        Q[:, 8 * t + 3] = qy2[sl]
        Q[:, 8 * t + 4] = s2dp[sl]                       # c2 bias (valid value)
        Q[:, 8 * t + 5] = (s2dp[sl] + 10.0) / 10.0        # c2 scale
        Q[:, 8 * t + 6] = kk[sl]                          # tensor_index k
    crit = int(criterion)
    alpha = 1.0 if crit in (-1, 0) else 0.0
    WUA = np.zeros((2, 128 * KT), np.float32)
    WUA[0, :] = alpha
    if crit in (-1, 1):
        WUA[1, :] = qap
    elif crit == 0:
        WUA[1, :] = 0.0
    else:
        WUA[1, :] = 1.0
    WON = np.ones((1, 128), np.float32)

    in_maps = []
    for c in range(NCORES):
        s = slice(c * NLOC, (c + 1) * NLOC)
        Ac = np.zeros((8, 2 * NLOC), np.float32)
        Ac[0, :NLOC] = boxes[s, 0]
        Ac[1, :NLOC] = boxes[s, 2]
        Ac[2, :NLOC] = boxes[s, 1]
        Ac[3, :NLOC] = boxes[s, 3]
        Ac[4, :NLOC] = barea[s]
        Ac[5, :NLOC] = 1.0
        Ac[6, 0::2] = s3d[s]
        Ac[6, 1::2] = dis[s]
        Ac[7, 1::2] = np.arange(c * NLOC, (c + 1) * NLOC, dtype=np.float32)
        in_maps.append({"A": Ac, "Q": Q, "WUA": WUA, "WON": WON})
    return in_maps


def run_cores(inputs, trace=False):
    """Compile (cached) + run on 8 cores; returns (results, BassKernelResults)."""
    from concourse.bass_utils import run_bass_kernel_spmd

    crit = int(inputs.get("criterion", -1))
    if crit not in _kernel_cache:
        _kernel_cache[crit] = _build_kernel(crit)
    nc = _kernel_cache[crit]
    in_maps = _host_prep(**inputs)
    res = run_bass_kernel_spmd(nc, in_maps, core_ids=list(range(NCORES)),
                               trace=trace)
    return res.results, res


def kernel(boxes, query_boxes, scores_3d, scores_2d, dis_to_lidar_3d,
           criterion=-1):
    results, _ = run_cores(dict(
        boxes=boxes, query_boxes=query_boxes, scores_3d=scores_3d,
        scores_2d=scores_2d, dis_to_lidar_3d=dis_to_lidar_3d,
        criterion=criterion))

    overlaps = np.empty((K, N, 4), np.float32)
    tensor_index = np.empty((K, N, 2), np.int32)
    valid = np.empty((K, N), bool)
    for c in range(NCORES):
        s = slice(c * NLOC, (c + 1) * NLOC)
        overlaps[:, s, :] = results[c]["OV"].reshape(K, NLOC, 4)
        tensor_index[:, s, :] = results[c]["IX"].reshape(K, NLOC, 2)
        valid[:, s] = results[c]["VA"].astype(bool)
    return (overlaps.reshape(K * N, 4), tensor_index.reshape(K * N, 2),
            valid.reshape(K * N))


# revision 6
# speedup vs baseline: 1.2795x; 1.2795x over previous
"""CLOCs pairwise-IoU association kernel for Trainium2 (8 NeuronCores).

Problem: boxes [N=10000,4], query_boxes [K=500,4] -> dense association tensor
  overlaps     [K*N, 4] f32 : (iou|-10, s3d, s2d|-10, dis) per (k,n) pair
  tensor_index [K*N, 2] i32 : (k, n)
  valid        [K*N]   bool : iw>0 & ih>0

Sharding: N split into 8 contiguous strips of 1250 (one per core); every core
holds all K=500 query boxes on the partition axis (4 k-tiles of <=128).
N-side per-box rows (coords/areas/scores/index) are partition-broadcast via
stride-0 DMA replication from HBM; K-side per-box values are per-partition
scalars fed to tensor_scalar / scalar_tensor_tensor ops.

Per k-tile dataflow (free dim = 1250 n's):
  DVE : t2=max(bx1,qx1); iw=min(bx2,qx2)-t2; t4=max(by1,qy1); ih=min(by2,qy2)-t4
        inter=max(iw,0)*relu(ih); ua0=barea+qarea; ua=ua0-inter; r~=1/ua
        vd=(inter<=0)*-10; out0=inter*r (+vd)
  ACT : relu(ih); out2=scale*vd+bias (c2 select); valid=sign(inter) u8
  POOL: tensor_index even lanes = k (per-partition const)
The (s3d,dis) output lanes and index odd lanes are constant across k-tiles:
baked once into ping-pong output tiles at setup, only lanes 0/2 rewritten.
Output DMAs are split into column chunks to spread across SDMA engines.
"""

import numpy as np

N = 10000
K = 500
NCORES = 8
NLOC = N // NCORES          # 1250
KT = 4                      # k-tiles of 128 (last has 116 real rows)
OV_CH = 5                   # overlaps DMA column chunks per k-tile
IX_CH = 2                   # index DMA column chunks per k-tile

_kernel_cache = {}


def _build_kernel(criterion: int):
    import concourse.bacc as bacc
    import concourse.tile as tile
    from concourse import mybir

    f32 = mybir.dt.float32
    i32 = mybir.dt.int32
    u8 = mybir.dt.uint8
    Alu = mybir.AluOpType
    Act = mybir.ActivationFunctionType

    nc = bacc.Bacc("TRN2", target_bir_lowering=False, debug=False,
                   num_devices=NCORES)

    A = nc.dram_tensor("A", [8, 2 * NLOC], f32, kind="ExternalInput").ap()
    IDXR = nc.dram_tensor("IDXR", [1, 2 * NLOC], i32, kind="ExternalInput").ap()
    Q = nc.dram_tensor("Q", [128, 8 * KT], f32, kind="ExternalInput").ap()

    OV = nc.dram_tensor("OV", [K, 4 * NLOC], f32, kind="ExternalOutput").ap()
    IX = nc.dram_tensor("IX", [K, 2 * NLOC], i32, kind="ExternalOutput").ap()
    VA = nc.dram_tensor("VA", [K, NLOC], u8, kind="ExternalOutput").ap()

    # criterion==-1 subtracts inter from the union denominator
    gamma = 1.0 if criterion == -1 else 0.0

    with tile.TileContext(nc) as tc:
        with (
            tc.tile_pool(name="const", bufs=1) as const,
            tc.tile_pool(name="tmp", bufs=1) as tmpp,
            tc.tile_pool(name="outv", bufs=2) as outv,
        ):
            Q_sb = const.tile([128, 8 * KT], f32)
            nc.sync.dma_start(out=Q_sb, in_=Q)

            # --- partition-broadcast n-side rows via stride-0 DMA ---
            BX1 = const.tile([128, NLOC], f32, tag="BX1")
            BX2 = const.tile([128, NLOC], f32, tag="BX2")
            BY1 = const.tile([128, NLOC], f32, tag="BY1")
            BY2 = const.tile([128, NLOC], f32, tag="BY2")
            BAREA = const.tile([128, NLOC], f32, tag="BAREA")
            for row, dst in ((0, BX1), (1, BX2), (2, BY1), (3, BY2),
                             (4, BAREA)):
                nc.gpsimd.dma_start(
                    out=dst, in_=A[row:row + 1, 0:NLOC].to_broadcast(
                        [128, NLOC]))

            # persistent ping-pong output tiles; (s3d,dis) lanes baked once
            Ts = [const.tile([128, NLOC, 4], f32, tag=f"T{i}", name=f"T{i}")
                  for i in range(2)]
            IDXs = [const.tile([128, NLOC, 2], i32, tag=f"IDX{i}",
                               name=f"IDX{i}") for i in range(2)]
            for idx_t in IDXs:
                nc.gpsimd.dma_start(
                    out=idx_t.rearrange("p f c -> p (f c)"),
                    in_=IDXR.to_broadcast([128, 2 * NLOC]))
            with tc.tile_pool(name="setup", bufs=1) as sp:
                S3DIS = sp.tile([128, 2 * NLOC], f32)
                nc.gpsimd.dma_start(
                    out=S3DIS, in_=A[6:7, :].to_broadcast([128, 2 * NLOC]))
                s3v = S3DIS.rearrange("p (f c) -> p f c", c=2)
                for T in Ts:
                    nc.scalar.activation(T[:, :, 1::2], s3v, Act.Copy)

            # --- steady: one pass per k-tile ---
            for t in range(KT):
                psz = 128 if t < KT - 1 else K - 128 * (KT - 1)
                T = Ts[t % 2]
                IDX = IDXs[t % 2]

                def q(j, t=t):
                    return Q_sb[:, 8 * t + j:8 * t + j + 1]

                t2 = tmpp.tile([128, NLOC], f32, tag="tm", bufs=2)
                t4 = tmpp.tile([128, NLOC], f32, tag="tm", bufs=2)
                iw = tmpp.tile([128, NLOC], f32, tag="iw", bufs=2)
                ih = tmpp.tile([128, NLOC], f32, tag="ih", bufs=2)
                ihp = tmpp.tile([128, NLOC], f32, tag="ihp", bufs=2)
                inter = tmpp.tile([128, NLOC], f32, tag="inter", bufs=2)
                ua0 = tmpp.tile([128, NLOC], f32, tag="ua0", bufs=2)
                ua = tmpp.tile([128, NLOC], f32, tag="ua", bufs=2)
                r = tmpp.tile([128, NLOC], f32, tag="r", bufs=2)
                vd = tmpp.tile([128, NLOC], f32, tag="vd", bufs=2)
                iou = tmpp.tile([128, NLOC], f32, tag="iou", bufs=2)
                V = outv.tile([128, NLOC], u8, tag="V")

                nc.vector.tensor_scalar_max(t2, BX1, q(0))
                nc.vector.scalar_tensor_tensor(iw, BX2, q(2), t2,
                                               Alu.min, Alu.subtract)
                nc.vector.tensor_scalar_max(t4, BY1, q(1))
                nc.vector.scalar_tensor_tensor(ih, BY2, q(3), t4,
                                               Alu.min, Alu.subtract)
                nc.scalar.activation(ihp, ih, Act.Relu)
                nc.vector.scalar_tensor_tensor(inter, iw, 0.0, ihp,
                                               Alu.max, Alu.mult)
                nc.vector.tensor_scalar_add(ua0, BAREA, q(7))
                nc.vector.scalar_tensor_tensor(ua, inter, -gamma, ua0,
                                               Alu.mult, Alu.add)
                nc.vector.reciprocal_approx_fast(r, ua)
                nc.vector.tensor_scalar(vd, inter, 0.0, -10.0,
                                        Alu.is_le, Alu.mult)
                nc.vector.scalar_tensor_tensor(iou, inter, 0.0, r,
                                               Alu.add, Alu.mult)
                nc.vector.scalar_tensor_tensor(T[:, :, 0], iou, 0.0, vd,
                                               Alu.add, Alu.add)
                nc.scalar.activation(T[:, :, 2], vd, Act.Identity,
                                     bias=q(4), scale=q(5))
                nc.scalar.activation(V, inter, Act.Sign)
                # tensor_index even lanes: per-partition k constant
                nc.gpsimd.tensor_scalar(IDX[:, :, 0], IDX[:, :, 1],
                                        0.0, q(6), Alu.mult, Alu.add)

                r0 = 128 * t
                ovf = T.rearrange("p f c -> p (f c)")
                cw = 4 * NLOC // OV_CH
                for ci in range(OV_CH):
                    nc.sync.dma_start(
                        out=OV[r0:r0 + psz, ci * cw:(ci + 1) * cw],
                        in_=ovf[:psz, ci * cw:(ci + 1) * cw])
                ixf = IDX.rearrange("p f c -> p (f c)")
                cwi = 2 * NLOC // IX_CH
                for ci in range(IX_CH):
                    nc.sync.dma_start(
                        out=IX[r0:r0 + psz, ci * cwi:(ci + 1) * cwi],
                        in_=ixf[:psz, ci * cwi:(ci + 1) * cwi])
                nc.sync.dma_start(out=VA[r0:r0 + psz, :], in_=V[:psz])

    nc.compile()
    return nc


def _host_prep(boxes, query_boxes, scores_3d, scores_2d, dis_to_lidar_3d,
               criterion):
    boxes = np.ascontiguousarray(boxes, dtype=np.float32)
    qb = np.ascontiguousarray(query_boxes, dtype=np.float32)
    s3d = np.ascontiguousarray(scores_3d, dtype=np.float32).reshape(-1)
    s2d = np.ascontiguousarray(scores_2d, dtype=np.float32).reshape(-1)
    dis = np.ascontiguousarray(dis_to_lidar_3d, dtype=np.float32).reshape(-1)

    barea = (boxes[:, 2] - boxes[:, 0]) * (boxes[:, 3] - boxes[:, 1])
    qarea = (qb[:, 2] - qb[:, 0]) * (qb[:, 3] - qb[:, 1])

    # K-side per-partition scalars, padded to 4*128 rows
    KP = 128 * KT
    qx1 = np.zeros(KP, np.float32); qy1 = np.zeros(KP, np.float32)
    qx2 = np.ones(KP, np.float32);  qy2 = np.ones(KP, np.float32)
    s2dp = np.zeros(KP, np.float32)
    qap = np.ones(KP, np.float32)
    qx1[:K], qy1[:K], qx2[:K], qy2[:K] = qb[:, 0], qb[:, 1], qb[:, 2], qb[:, 3]
    s2dp[:K] = s2d
    qap[:K] = qarea

    crit = int(criterion)
    # union term: ua = ua0 - gamma*inter, ua0 = alpha*barea + w[k]
    if crit == -1:
        wk = qap
    elif crit == 0:
        wk = np.zeros(KP, np.float32)
    elif crit == 1:
        wk = qap
    else:
        wk = np.ones(KP, np.float32)

    Q = np.zeros((128, 8 * KT), np.float32)
    kk = np.arange(KP, dtype=np.float32)
    for t in range(KT):
        sl = slice(128 * t, 128 * (t + 1))
        Q[:, 8 * t + 0] = qx1[sl]
        Q[:, 8 * t + 1] = qy1[sl]
        Q[:, 8 * t + 2] = qx2[sl]
        Q[:, 8 * t + 3] = qy2[sl]
        Q[:, 8 * t + 4] = s2dp[sl]                        # c2 bias (valid)
        Q[:, 8 * t + 5] = (s2dp[sl] + 10.0) / 10.0        # c2 scale
        Q[:, 8 * t + 6] = kk[sl]                          # tensor_index k
        Q[:, 8 * t + 7] = wk[sl]                          # ua0 k-term
    alpha = 1.0 if crit in (-1, 0) else 0.0

    in_maps = []
    for c in range(NCORES):
        s = slice(c * NLOC, (c + 1) * NLOC)
        Ac = np.zeros((8, 2 * NLOC), np.float32)
        Ac[0, :NLOC] = boxes[s, 0]
        Ac[1, :NLOC] = boxes[s, 2]
        Ac[2, :NLOC] = boxes[s, 1]
        Ac[3, :NLOC] = boxes[s, 3]
        Ac[4, :NLOC] = alpha * barea[s]
        Ac[6, 0::2] = s3d[s]
        Ac[6, 1::2] = dis[s]
        idxr = np.zeros((1, 2 * NLOC), np.int32)
        idxr[0, 1::2] = np.arange(c * NLOC, (c + 1) * NLOC, dtype=np.int32)
        in_maps.append({"A": Ac, "IDXR": idxr, "Q": Q})
    return in_maps


def run_cores(inputs, trace=False):
    """Compile (cached) + run on 8 cores; returns (results, BassKernelResults)."""
    from concourse.bass_utils import run_bass_kernel_spmd

    crit = int(inputs.get("criterion", -1))
    if crit not in _kernel_cache:
        _kernel_cache[crit] = _build_kernel(crit)
    nc = _kernel_cache[crit]
    in_maps = _host_prep(**inputs)
    res = run_bass_kernel_spmd(nc, in_maps, core_ids=list(range(NCORES)),
                               trace=trace)
    return res.results, res


def kernel(boxes, query_boxes, scores_3d, scores_2d, dis_to_lidar_3d,
           criterion=-1):
    results, _ = run_cores(dict(
        boxes=boxes, query_boxes=query_boxes, scores_3d=scores_3d,
        scores_2d=scores_2d, dis_to_lidar_3d=dis_to_lidar_3d,
        criterion=criterion))

    overlaps = np.empty((K, N, 4), np.float32)
    tensor_index = np.empty((K, N, 2), np.int32)
    valid = np.empty((K, N), bool)
    for c in range(NCORES):
        s = slice(c * NLOC, (c + 1) * NLOC)
        overlaps[:, s, :] = results[c]["OV"].reshape(K, NLOC, 4)
        tensor_index[:, s, :] = results[c]["IX"].reshape(K, NLOC, 2)
        valid[:, s] = results[c]["VA"].astype(bool)
    return (overlaps.reshape(K * N, 4), tensor_index.reshape(K * N, 2),
            valid.reshape(K * N))


# revision 13
# speedup vs baseline: 1.6452x; 1.2858x over previous
"""CLOCs pairwise-IoU association kernel for Trainium2 (8 NeuronCores).

Problem: boxes [N=10000,4], query_boxes [K=500,4] -> dense association tensor
  overlaps     [K*N, 4] f32 : (iou|-10, s3d, s2d|-10, dis) per (k,n) pair
  tensor_index [K*N, 2] i32 : (k, n)
  valid        [K*N]   bool : iw>0 & ih>0

Sharding: N split into 8 contiguous strips of 1250 (one per core); every core
holds all K=500 query boxes on the partition axis (4 k-tiles of <=128).
N-side per-box rows (coords/areas/scores/index) are partition-broadcast via
stride-0 DMA replication from HBM; K-side per-box values are per-partition
scalars fed to tensor_scalar / scalar_tensor_tensor ops.

Per k-tile dataflow (free dim = 1250 n's):
  DVE : t2=max(bx1,qx1); iw=min(bx2,qx2)-t2; t4=max(by1,qy1); ih=min(by2,qy2)-t4
        inter=max(iw,0)*relu(ih); ua0=barea+qarea; ua=ua0-inter; r~=1/ua
        vd=(inter<=0)*-10; out0=inter*r (+vd)
  ACT : relu(ih); out2=scale*vd+bias (c2 select); valid=sign(inter) u8
  POOL: tensor_index even lanes = k (per-partition const)
The (s3d,dis) output lanes and index odd lanes are constant across k-tiles:
baked once into ping-pong output tiles at setup, only lanes 0/2 rewritten.
Output DMAs are split into column chunks to spread across SDMA engines.
"""

import numpy as np

N = 10000
K = 500
KP = 512                    # K padded to 4*128 so every DMA is 128 partitions
NCORES = 8
NLOC = N // NCORES          # 1250
KT = 4                      # k-tiles of 128
OV_CH = 5                   # overlaps DMA column chunks per k-tile
IX_CH = 2                   # index DMA column chunks per k-tile

_kernel_cache = {}


def _build_kernel(criterion: int):
    import concourse.bacc as bacc
    import concourse.tile as tile
    from concourse import mybir

    f32 = mybir.dt.float32
    i32 = mybir.dt.int32
    u8 = mybir.dt.uint8
    Alu = mybir.AluOpType
    Act = mybir.ActivationFunctionType

    nc = bacc.Bacc("TRN2", target_bir_lowering=False, debug=False,
                   num_devices=NCORES)

    A = nc.dram_tensor("A", [8, 2 * NLOC], f32, kind="ExternalInput").ap()
    IDXR = nc.dram_tensor("IDXR", [1, 2 * NLOC], i32, kind="ExternalInput").ap()
    Q = nc.dram_tensor("Q", [128, 8 * KT], f32, kind="ExternalInput").ap()

    OV = nc.dram_tensor("OV", [KP, 4 * NLOC], f32, kind="ExternalOutput").ap()
    IX = nc.dram_tensor("IX", [KP, 2 * NLOC], i32, kind="ExternalOutput").ap()
    VA = nc.dram_tensor("VA", [KP, NLOC], u8, kind="ExternalOutput").ap()

    # criterion==-1 subtracts inter from the union denominator
    gamma = 1.0 if criterion == -1 else 0.0

    with tile.TileContext(nc) as tc:
        with (
            tc.tile_pool(name="const", bufs=1) as const,
            tc.tile_pool(name="tmp", bufs=1) as tmpp,
            tc.tile_pool(name="outv", bufs=2) as outv,
        ):
            Q_sb = const.tile([128, 8 * KT], f32)
            nc.sync.dma_start(out=Q_sb, in_=Q)

            # --- partition-broadcast n-side rows via stride-0 DMA ---
            BX1 = const.tile([128, NLOC], f32, tag="BX1")
            BX2 = const.tile([128, NLOC], f32, tag="BX2")
            BY1 = const.tile([128, NLOC], f32, tag="BY1")
            BY2 = const.tile([128, NLOC], f32, tag="BY2")
            BAREA = const.tile([128, NLOC], f32, tag="BAREA")
            for row, dst in ((0, BX1), (1, BX2), (2, BY1), (3, BY2),
                             (4, BAREA)):
                nc.gpsimd.dma_start(
                    out=dst, in_=A[row:row + 1, 0:NLOC].to_broadcast(
                        [128, NLOC]))

            # persistent ping-pong output tiles; (s3d,dis) lanes baked once
            Ts = [const.tile([128, NLOC, 4], f32, tag=f"T{i}", name=f"T{i}")
                  for i in range(2)]
            IDXs = [const.tile([128, NLOC, 2], i32, tag=f"IDX{i}",
                               name=f"IDX{i}") for i in range(2)]
            for idx_t in IDXs:
                nc.gpsimd.dma_start(
                    out=idx_t.rearrange("p f c -> p (f c)"),
                    in_=IDXR.to_broadcast([128, 2 * NLOC]))
            with tc.tile_pool(name="setup", bufs=1) as sp:
                S3DIS = sp.tile([128, 2 * NLOC], f32)
                nc.gpsimd.dma_start(
                    out=S3DIS, in_=A[6:7, :].to_broadcast([128, 2 * NLOC]))
                s3v = S3DIS.rearrange("p (f c) -> p f c", c=2)
                for T in Ts:
                    nc.scalar.activation(T[:, :, 1::2], s3v, Act.Copy)

            # --- steady: one pass per k-tile ---
            for t in range(KT):
                psz = 128
                T = Ts[t % 2]
                IDX = IDXs[t % 2]

                def q(j, t=t):
                    return Q_sb[:, 8 * t + j:8 * t + j + 1]

                t2 = tmpp.tile([128, NLOC], f32, tag="tm", bufs=2)
                t4 = tmpp.tile([128, NLOC], f32, tag="tm", bufs=2)
                iw = tmpp.tile([128, NLOC], f32, tag="iw", bufs=2)
                ih = tmpp.tile([128, NLOC], f32, tag="ih", bufs=2)
                ihp = tmpp.tile([128, NLOC], f32, tag="ihp", bufs=2)
                inter = tmpp.tile([128, NLOC], f32, tag="inter", bufs=2)
                ua0 = tmpp.tile([128, NLOC], f32, tag="ua0", bufs=2)
                ua = tmpp.tile([128, NLOC], f32, tag="ua", bufs=2)
                r = tmpp.tile([128, NLOC], f32, tag="r", bufs=2)
                vd = tmpp.tile([128, NLOC], f32, tag="vd", bufs=2)
                iou = tmpp.tile([128, NLOC], f32, tag="iou", bufs=2)
                V = outv.tile([128, NLOC], u8, tag="V")

                nc.vector.tensor_scalar_max(t2, BX1, q(0))
                nc.vector.scalar_tensor_tensor(iw, BX2, q(2), t2,
                                               Alu.min, Alu.subtract)
                nc.vector.tensor_scalar_max(t4, BY1, q(1))
                nc.vector.scalar_tensor_tensor(ih, BY2, q(3), t4,
                                               Alu.min, Alu.subtract)
                nc.scalar.activation(ihp, ih, Act.Relu)
                nc.vector.scalar_tensor_tensor(inter, iw, 0.0, ihp,
                                               Alu.max, Alu.mult)
                nc.scalar.activation(ua0, BAREA, Act.Identity,
                                     bias=q(7), scale=1.0)
                nc.vector.scalar_tensor_tensor(ua, inter, -gamma, ua0,
                                               Alu.mult, Alu.add)
                nc.vector.reciprocal_approx_fast(r, ua)
                # vd = 1.0 where invalid (inter<=0), else 0.0
                nc.vector.tensor_scalar(vd, inter, 0.0, None, Alu.is_le)
                nc.vector.scalar_tensor_tensor(iou, inter, 0.0, r,
                                               Alu.add, Alu.mult)
                nc.vector.scalar_tensor_tensor(T[:, :, 0], vd, -10.0, iou,
                                               Alu.mult, Alu.add)
                nc.scalar.activation(T[:, :, 2], vd, Act.Identity,
                                     bias=q(4), scale=q(5))
                nc.scalar.activation(V, inter, Act.Sign)
                # tensor_index even lanes: per-partition k constant
                nc.gpsimd.tensor_scalar(IDX[:, :, 0], IDX[:, :, 1],
                                        0.0, q(6), Alu.mult, Alu.add)

                r0 = 128 * t
                ovf = T.rearrange("p f c -> p (f c)")
                cw = 4 * NLOC // OV_CH
                for ci in range(OV_CH):
                    nc.sync.dma_start(
                        out=OV[r0:r0 + psz, ci * cw:(ci + 1) * cw],
                        in_=ovf[:psz, ci * cw:(ci + 1) * cw])
                ixf = IDX.rearrange("p f c -> p (f c)")
                cwi = 2 * NLOC // IX_CH
                for ci in range(IX_CH):
                    nc.sync.dma_start(
                        out=IX[r0:r0 + psz, ci * cwi:(ci + 1) * cwi],
                        in_=ixf[:psz, ci * cwi:(ci + 1) * cwi])
                nc.sync.dma_start(out=VA[r0:r0 + psz, :], in_=V[:psz])

    nc.compile()
    return nc


def _host_prep(boxes, query_boxes, scores_3d, scores_2d, dis_to_lidar_3d,
               criterion):
    boxes = np.ascontiguousarray(boxes, dtype=np.float32)
    qb = np.ascontiguousarray(query_boxes, dtype=np.float32)
    s3d = np.ascontiguousarray(scores_3d, dtype=np.float32).reshape(-1)
    s2d = np.ascontiguousarray(scores_2d, dtype=np.float32).reshape(-1)
    dis = np.ascontiguousarray(dis_to_lidar_3d, dtype=np.float32).reshape(-1)

    barea = (boxes[:, 2] - boxes[:, 0]) * (boxes[:, 3] - boxes[:, 1])
    qarea = (qb[:, 2] - qb[:, 0]) * (qb[:, 3] - qb[:, 1])

    # K-side per-partition scalars, padded to 4*128 rows
    qx1 = np.zeros(KP, np.float32); qy1 = np.zeros(KP, np.float32)
    qx2 = np.ones(KP, np.float32);  qy2 = np.ones(KP, np.float32)
    s2dp = np.zeros(KP, np.float32)
    qap = np.ones(KP, np.float32)
    qx1[:K], qy1[:K], qx2[:K], qy2[:K] = qb[:, 0], qb[:, 1], qb[:, 2], qb[:, 3]
    s2dp[:K] = s2d
    qap[:K] = qarea

    crit = int(criterion)
    # union term: ua = ua0 - gamma*inter, ua0 = alpha*barea + w[k]
    if crit == -1:
        wk = qap
    elif crit == 0:
        wk = np.zeros(KP, np.float32)
    elif crit == 1:
        wk = qap
    else:
        wk = np.ones(KP, np.float32)

    Q = np.zeros((128, 8 * KT), np.float32)
    kk = np.arange(KP, dtype=np.float32)
    for t in range(KT):
        sl = slice(128 * t, 128 * (t + 1))
        Q[:, 8 * t + 0] = qx1[sl]
        Q[:, 8 * t + 1] = qy1[sl]
        Q[:, 8 * t + 2] = qx2[sl]
        Q[:, 8 * t + 3] = qy2[sl]
        Q[:, 8 * t + 4] = s2dp[sl]                        # c2 bias (valid)
        Q[:, 8 * t + 5] = -10.0 - s2dp[sl]                # c2 scale (vd=1)
        Q[:, 8 * t + 6] = kk[sl]                          # tensor_index k
        Q[:, 8 * t + 7] = wk[sl]                          # ua0 k-term
    alpha = 1.0 if crit in (-1, 0) else 0.0

    in_maps = []
    for c in range(NCORES):
        s = slice(c * NLOC, (c + 1) * NLOC)
        Ac = np.zeros((8, 2 * NLOC), np.float32)
        Ac[0, :NLOC] = boxes[s, 0]
        Ac[1, :NLOC] = boxes[s, 2]
        Ac[2, :NLOC] = boxes[s, 1]
        Ac[3, :NLOC] = boxes[s, 3]
        Ac[4, :NLOC] = alpha * barea[s]
        Ac[6, 0::2] = s3d[s]
        Ac[6, 1::2] = dis[s]
        idxr = np.zeros((1, 2 * NLOC), np.int32)
        idxr[0, 1::2] = np.arange(c * NLOC, (c + 1) * NLOC, dtype=np.int32)
        in_maps.append({"A": Ac, "IDXR": idxr, "Q": Q})
    return in_maps


def run_cores(inputs, trace=False):
    """Compile (cached) + run on 8 cores; returns (results, BassKernelResults)."""
    from concourse.bass_utils import run_bass_kernel_spmd

    crit = int(inputs.get("criterion", -1))
    if crit not in _kernel_cache:
        _kernel_cache[crit] = _build_kernel(crit)
    nc = _kernel_cache[crit]
    in_maps = _host_prep(**inputs)
    res = run_bass_kernel_spmd(nc, in_maps, core_ids=list(range(NCORES)),
                               trace=trace)
    return res.results, res


def kernel(boxes, query_boxes, scores_3d, scores_2d, dis_to_lidar_3d,
           criterion=-1):
    results, _ = run_cores(dict(
        boxes=boxes, query_boxes=query_boxes, scores_3d=scores_3d,
        scores_2d=scores_2d, dis_to_lidar_3d=dis_to_lidar_3d,
        criterion=criterion))

    overlaps = np.empty((K, N, 4), np.float32)
    tensor_index = np.empty((K, N, 2), np.int32)
    valid = np.empty((K, N), bool)
    for c in range(NCORES):
        s = slice(c * NLOC, (c + 1) * NLOC)
        overlaps[:, s, :] = results[c]["OV"].reshape(KP, NLOC, 4)[:K]
        tensor_index[:, s, :] = results[c]["IX"].reshape(KP, NLOC, 2)[:K]
        valid[:, s] = results[c]["VA"].astype(bool)[:K]
    return (overlaps.reshape(K * N, 4), tensor_index.reshape(K * N, 2),
            valid.reshape(K * N))
